# revision 15
# baseline (speedup 1.0000x reference)
# Binarized 3x3 conv (per-direction / population-parallel), Trainium2 Bass kernel.
#
# Reference math: bits {0,1} -> {-1,+1}; out = 4*xw - 2*sx - 2*sw + K.
# Identity used here:  out = conv(x, W4) - T2
#   where W4 = 4w - 2 (values +-2, exact in fp8e4), T2[cout] = sum (2w-1),
#   conv is a standard zero-padded 3x3 conv with x in {0,1}.
# Proof: sum(x*(4w-2)) - sum(2w-1) = 4xw - 2sx - (2sw - K).
# Output values are integers in [-1152, 1152] -> exact in fp16.
#
# Sharding: D=64 directions split 8 per core across 8 NeuronCores (pure
# population parallelism, no communication).
#
# All data conditioning happens on the host (not part of the HW kernel).
# Per direction one combined 2312 B/partition buffer is uploaded:
#   [0:1152]    W4 as fp8, taps permuted so DoubleRow pairs are adjacent
#   [1152:2308] zero-padded channel-major fp8 {0,1} image [34, 34]
#   [2308:2312] -T2 bias as f32 (partition = cout)
# so each direction is a single DMA (descriptor generation is the fill
# bottleneck, ~0.7us per DMA instruction).
#
# The device runs the conv as fp8 DoubleRow matmuls: two taps per matmul
# (2 fp8 weights per PE cell, 2x throughput), 4 pairs + 1 normal tap per
# 512-pixel block, accumulating [cout, pix] in PSUM.  The rhs pair planes
# are raw 4D access patterns over the padded image (pair stride = tap
# offset delta).  Per block: epilogue adds -T2 (ACT for block 0, DVE for
# block 1) into fp16 and DMAs out, so the block-0 epilogue hides under
# block 1's matmuls.  Scratch warmup matmuls run during the DMA fill so
# the PE clock gate (HAM, 1.2 -> 2.4 GHz after ~3.4us busy) is warm when
# the real work starts.  The host transposes [cout, pix] fp16 back to
# [pix, cout] f32 (exact, integer values).

import numpy as np

import concourse.bass as bass
import concourse.mybir as mybir
import concourse.tile as tile
from concourse import bacc
from concourse import bass_utils

N_CORES = 8
D, H, W, CIN, COUT = 64, 32, 32, 128, 128
DPC = D // N_CORES  # directions per core
NPIX = H * W  # 1024
IMH, IMW = 34, 34  # padded image
IMSZ = IMH * IMW  # 1156
WSZ = 9 * COUT  # 1152
CSZ = WSZ + IMSZ + 4  # 2312 combined bytes/partition/direction

FP32 = mybir.dt.float32
FP16 = mybir.dt.float16
BF16 = mybir.dt.bfloat16
FP8 = mybir.dt.float8e4
I8 = mybir.dt.int8

ONE_FP8 = 0x38  # 1.0 in e4m3
POS2_FP8 = 0x40  # 2.0
NEG2_FP8 = 0xC0  # -2.0

# Tap order in the uploaded weight buffer: DoubleRow pairs adjacent.
# (i, j) = (filter row, filter col); window offset in image = i*34 + j.
TAP_PERM = [(0, 0), (0, 1), (1, 0), (1, 1), (2, 0), (2, 1), (0, 2), (1, 2), (2, 2)]
N_WARMUP = 6


def _body(nc, tc, in_d, t_d, o_d):
    Act = mybir.ActivationFunctionType
    Alu = mybir.AluOpType
    DR = mybir.MatmulPerfMode.DoubleRow
    with (
        tc.tile_pool(name="const", bufs=1) as constp,
        tc.tile_pool(name="of", bufs=2 * DPC, space="SBUF") as ofp,
        tc.tile_pool(name="psA", bufs=4, space="PSUM") as psA,
        tc.tile_pool(name="psW", bufs=1, space="PSUM") as psW,
    ):
        # PE warmup during the DMA fill (memset on gpsimd: its queue is
        # ready ~1.5us before the vector engine's).
        scratch = constp.tile([128, 512], BF16)
        nc.gpsimd.memset(scratch, 0.0)
        wacc = psW.tile([128, 512], FP32)
        for _ in range(N_WARMUP):
            nc.tensor.matmul(
                wacc, lhsT=scratch[:, 0:128], rhs=scratch, start=True, stop=True
            )

        # Input DMAs, all issued upfront (SBUF holds every direction).
        # Direction 0 split across the three DGE queues, ordered so the
        # first matmuls' operands (weights, then image rows) land first.
        inall = constp.tile([128, DPC, CSZ], I8)
        negT = constp.tile([128, DPC], FP32)
        nc.gpsimd.dma_start(negT, t_d)
        nc.sync.dma_start(inall[:, 0, 0:WSZ], in_d[0][:, 0:WSZ])
        nc.scalar.dma_start(
            inall[:, 0, WSZ : WSZ + 17 * IMW], in_d[0][:, WSZ : WSZ + 17 * IMW]
        )
        nc.gpsimd.dma_start(
            inall[:, 0, WSZ + 17 * IMW :], in_d[0][:, WSZ + 17 * IMW :]
        )
        for d in range(1, DPC):
            q = nc.sync if d % 2 else nc.scalar
            q.dma_start(inall[:, d], in_d[d])

        for d in range(DPC):
            base = inall[:, d].bitcast(FP8)
            pstride = base.ap[0]
            wsl = inall[:, d, 0:WSZ].rearrange("p (t o) -> p t o", t=9)
            bias = negT[:, d : d + 1]
            od = o_d[d].rearrange("c (b n) -> c b n", b=2)
            # 9-tap conv: out[cout, pix] += W4[tap].T @ image[window], as 4
            # DoubleRow pair-matmuls + 1 normal per 512-pixel block.  The
            # rhs pair AP reads both taps' windows (2nd plane at +delta).
            for b in range(2):
                ob = psA.tile([128, 512], FP32, tag="acc", name=f"acc{d}{b}")
                for k in range(4):
                    (i0, j0), (i1, j1) = TAP_PERM[2 * k], TAP_PERM[2 * k + 1]
                    off = WSZ + (16 * b + i0) * IMW + j0
                    delta = (i1 - i0) * IMW + (j1 - j0)
                    rhs = bass.AP(
                        base.tensor,
                        base.offset + off,
                        [pstride, [delta, 2], [IMW, 16], [1, 32]],
                    )
                    nc.tensor.matmul(
                        ob,
                        lhsT=wsl[:, 2 * k : 2 * k + 2, :].bitcast(FP8),
                        rhs=rhs,
                        start=(k == 0), stop=False, perf_mode=DR,
                    )
                i8, j8 = TAP_PERM[8]
                off = WSZ + (16 * b + i8) * IMW + j8
                rhs = bass.AP(
                    base.tensor, base.offset + off, [pstride, [IMW, 16], [1, 32]]
                )
                nc.tensor.matmul(
                    ob, lhsT=wsl[:, 8, :].bitcast(FP8), rhs=rhs,
                    start=False, stop=True
                )
                # Epilogue: out = acc - T2, fp16 (exact: integers <= 1152).
                last = d == DPC - 1
                if not (last and b == 1):
                    ofb = ofp.tile([128, 512], FP16, tag="of", name=f"of{d}{b}")
                    if b == 0:
                        nc.scalar.activation(
                            ofb, ob, Act.Identity, bias=bias, scale=1.0
                        )
                        nc.gpsimd.dma_start(od[:, b], ofb)
                    else:
                        nc.vector.tensor_scalar(
                            ofb, ob, 1.0, bias, Alu.mult, Alu.add
                        )
                        nc.sync.dma_start(od[:, b], ofb)
                else:
                    # Final block: quarters on both engines, halves on both
                    # DMA queues, to shorten the drain tail.
                    oq = od[:, 1].rearrange("c (q n) -> c q n", q=2)
                    for q in range(2):
                        ofq = ofp.tile(
                            [128, 256], FP16, tag="of", name=f"oflast{q}"
                        )
                        sl = slice(256 * q, 256 * (q + 1))
                        if q == 0:
                            nc.scalar.activation(
                                ofq, ob[:, sl], Act.Identity, bias=bias, scale=1.0
                            )
                            nc.gpsimd.dma_start(oq[:, q], ofq)
                        else:
                            nc.vector.tensor_scalar(
                                ofq, ob[:, sl], 1.0, bias, Alu.mult, Alu.add
                            )
                            nc.sync.dma_start(oq[:, q], ofq)


_NC_CACHE = None


def _get_nc():
    global _NC_CACHE
    if _NC_CACHE is None:
        nc = bacc.Bacc(
            "TRN2", target_bir_lowering=False, debug=False, num_devices=N_CORES
        )
        in_d = nc.dram_tensor(
            "in_s", [DPC, CIN, CSZ], I8, kind="ExternalInput"
        ).ap()
        t_d = nc.dram_tensor("t_s", [COUT, DPC], FP32, kind="ExternalInput").ap()
        o_d = nc.dram_tensor(
            "out_s", [DPC, COUT, NPIX], FP16, kind="ExternalOutput"
        ).ap()
        with tile.TileContext(nc) as tc:
            _body(nc, tc, in_d, t_d, o_d)
        nc.compile()
        _NC_CACHE = nc
    return _NC_CACHE


def _in_maps(x, w):
    # x: [D,H,W,CIN] bool -> zero-padded channel-major fp8 {0,1} image.
    xb = np.ascontiguousarray(x).view(np.uint8)  # 0/1
    xim = np.zeros((D, CIN, IMH, IMW), np.uint8)
    xim[:, :, 1 : H + 1, 1 : W + 1] = (
        np.transpose(xb, (0, 3, 1, 2)) * np.uint8(ONE_FP8)
    )

    # w: [D,3,3,CIN,COUT] f32 {0,1} -> fp8 W4 = 4w-2 in [cin, tap, cout],
    # taps ordered per TAP_PERM (DoubleRow pairs adjacent).
    wb = np.ascontiguousarray(w) > 0.5
    w4 = np.where(wb, np.uint8(POS2_FP8), np.uint8(NEG2_FP8))
    perm = [3 * i + j for (i, j) in TAP_PERM]
    w4 = np.ascontiguousarray(
        np.transpose(w4.reshape(D, 9, CIN, COUT)[:, perm], (0, 2, 1, 3))
    )

    # -T2[cout] = -(2*sum(w) - K); stored per partition = cout.
    sw = wb.sum(axis=(1, 2, 3), dtype=np.int32)  # [D, COUT]
    negT = (9 * CIN - 2 * sw).astype(np.float32)  # [D, COUT]

    comb = np.empty((D, CIN, CSZ), np.uint8)
    comb[:, :, 0:WSZ] = w4.reshape(D, CIN, WSZ)
    comb[:, :, WSZ : WSZ + IMSZ] = xim.reshape(D, CIN, IMSZ)
    comb[:, :, WSZ + IMSZ :] = negT.view(np.uint8).reshape(D, COUT, 4)
    comb = comb.view(np.int8)

    negTT = np.ascontiguousarray(negT.T)  # [COUT, D]
    return [
        {
            "in_s": comb[c * DPC : (c + 1) * DPC],
            "t_s": negTT[:, c * DPC : (c + 1) * DPC],
        }
        for c in range(N_CORES)
    ]


def kernel(x, w, _trace=False):
    nc = _get_nc()
    res = bass_utils.run_bass_kernel_spmd(
        nc, _in_maps(x, w), core_ids=list(range(N_CORES)), trace=_trace
    )
    out = np.concatenate([r["out_s"] for r in res.results], axis=0)
    # [D, COUT, NPIX] fp16 -> [D, H, W, COUT] f32 (exact: integer values)
    out = np.transpose(out, (0, 2, 1)).reshape(D, H, W, COUT).astype(np.float32)
    if _trace:
        return out, res
    return out


# revision 16
# speedup vs baseline: 1.1040x; 1.1040x over previous
# Binarized 3x3 conv (per-direction / population-parallel), Trainium2 Bass kernel.
#
# Reference math: bits {0,1} -> {-1,+1}; out = 4*xw - 2*sx - 2*sw + K.
# Identity used here:  out = conv(x, W4) - T2
#   where W4 = 4w - 2 (values +-2, exact in fp8e4), T2[cout] = sum (2w-1),
#   conv is a standard zero-padded 3x3 conv with x in {0,1}.
# Proof: sum(x*(4w-2)) - sum(2w-1) = 4xw - 2sx - (2sw - K).
# Output values are integers in [-1152, 1152] -> exact in fp16.
#
# Sharding: D=64 directions split 8 per core across 8 NeuronCores (pure
# population parallelism, no communication).
#
# All data conditioning happens on the host (not part of the HW kernel):
# x is uploaded as a zero-padded channel-major fp8 {0,1} image [34, 34],
# w as fp8 W4 = 4w-2 in [cin, tap, cout] with taps permuted so DoubleRow
# pairs are adjacent, and the -T2 bias as f32 [cout, DPC].  (Keeping x/w
# in separate SBUF tiles matters: it keeps the Tile framework's subtile
# dependency tracking precise, so the first matmuls start as soon as
# direction 0's data lands instead of waiting for every input DMA.)
#
# The device runs the conv as fp8 DoubleRow matmuls: two taps per matmul
# (2 fp8 weights per PE cell, 2x throughput), 4 pairs + 1 normal tap per
# 512-pixel block, accumulating [cout, pix] in PSUM.  The rhs pair planes
# are raw 4D access patterns over the padded image (pair stride = tap
# offset delta).  Per block: epilogue adds -T2 (ACT for block 0, DVE for
# block 1) into fp16 and DMAs out, so the block-0 epilogue hides under
# block 1's matmuls.  Scratch warmup matmuls run during the DMA fill so
# the PE clock gate (HAM, 1.2 -> 2.4 GHz after ~3.4us sustained busy) is
# warm when the real work starts.  The host transposes [cout, pix] fp16
# back to [pix, cout] f32 (exact, integer values).

import numpy as np

import concourse.bass as bass
import concourse.mybir as mybir
import concourse.tile as tile
from concourse import bacc
from concourse import bass_utils

N_CORES = 8
D, H, W, CIN, COUT = 64, 32, 32, 128, 128
DPC = D // N_CORES  # directions per core
NPIX = H * W  # 1024
IMH, IMW = 34, 34  # padded image
IMSZ = IMH * IMW  # 1156
WSZ = 9 * COUT  # 1152

FP32 = mybir.dt.float32
FP16 = mybir.dt.float16
BF16 = mybir.dt.bfloat16
FP8 = mybir.dt.float8e4
I8 = mybir.dt.int8

ONE_FP8 = 0x38  # 1.0 in e4m3
POS2_FP8 = 0x40  # 2.0
NEG2_FP8 = 0xC0  # -2.0

# Tap order in the uploaded weight buffer: DoubleRow pairs adjacent.
# (i, j) = (filter row, filter col); window offset in image = i*34 + j.
TAP_PERM = [(0, 0), (0, 1), (1, 0), (1, 1), (2, 0), (2, 1), (0, 2), (1, 2), (2, 2)]
N_WARMUP = 5


def _body(nc, tc, x_d, w_d, t_d, o_d):
    Act = mybir.ActivationFunctionType
    Alu = mybir.AluOpType
    DR = mybir.MatmulPerfMode.DoubleRow
    with (
        tc.tile_pool(name="const", bufs=1) as constp,
        tc.tile_pool(name="of", bufs=2 * DPC, space="SBUF") as ofp,
        tc.tile_pool(name="psA", bufs=4, space="PSUM") as psA,
        tc.tile_pool(name="psW", bufs=1, space="PSUM") as psW,
    ):
        # PE warmup: HAM un-throttles (1.2 -> 2.4 GHz) only after ~3.4us of
        # sustained matmul activity; burn the DMA-fill window on scratch
        # matmuls so the real ones run warm.
        scratch = constp.tile([128, 512], BF16)
        nc.vector.memset(scratch, 0.0)
        wacc = psW.tile([128, 512], FP32)
        for _ in range(N_WARMUP):
            nc.tensor.matmul(
                wacc, lhsT=scratch[:, 0:128], rhs=scratch, start=True, stop=True
            )

        # All input DMAs issued upfront (SBUF easily fits every direction).
        # Direction 0 is split into halves across the two HWDGE queues
        # (sync/scalar) so its descgen + transfer lands soonest; gpsimd's
        # queue starts later, so it only carries the tiny bias load and
        # the block-0 output DMAs.
        xall = constp.tile([128, DPC, IMSZ], I8)
        wall = constp.tile([128, DPC, 9, COUT], I8)
        negT = constp.tile([128, DPC], FP32)
        x0 = x_d[0].rearrange("c h w -> c (h w)")
        w0 = w_d[0].rearrange("c t o -> c (t o)")
        x0sb = xall[:, 0]
        w0sb = wall[:, 0].rearrange("p t o -> p (t o)")
        nc.sync.dma_start(x0sb[:, 0:578], x0[:, 0:578])
        nc.scalar.dma_start(w0sb[:, 0:576], w0[:, 0:576])
        nc.sync.dma_start(w0sb[:, 576:1152], w0[:, 576:1152])
        nc.scalar.dma_start(x0sb[:, 578:1156], x0[:, 578:1156])
        nc.gpsimd.dma_start(negT, t_d)
        for d in range(1, DPC):
            nc.sync.dma_start(xall[:, d], x_d[d].rearrange("c h w -> c (h w)"))
            nc.scalar.dma_start(wall[:, d], w_d[d])

        for d in range(DPC):
            xim = xall[:, d].bitcast(FP8)
            pstride = xim.ap[0]
            bias = negT[:, d : d + 1]
            od = o_d[d].rearrange("c (b n) -> c b n", b=2)
            # 9-tap conv: out[cout, pix] += W4[tap].T @ xim[window], as 4
            # DoubleRow pair-matmuls + 1 normal per 512-pixel block.  The
            # rhs pair AP reads both taps' windows (2nd plane at +delta).
            for b in range(2):
                ob = psA.tile([128, 512], FP32, tag="acc", name=f"acc{d}{b}")
                for k in range(4):
                    (i0, j0), (i1, j1) = TAP_PERM[2 * k], TAP_PERM[2 * k + 1]
                    off = (16 * b + i0) * IMW + j0
                    delta = (i1 - i0) * IMW + (j1 - j0)
                    rhs = bass.AP(
                        xim.tensor,
                        xim.offset + off,
                        [pstride, [delta, 2], [IMW, 16], [1, 32]],
                    )
                    nc.tensor.matmul(
                        ob,
                        lhsT=wall[:, d, 2 * k : 2 * k + 2, :].bitcast(FP8),
                        rhs=rhs,
                        start=(k == 0), stop=False, perf_mode=DR,
                    )
                i8, j8 = TAP_PERM[8]
                off = (16 * b + i8) * IMW + j8
                rhs = bass.AP(
                    xim.tensor, xim.offset + off, [pstride, [IMW, 16], [1, 32]]
                )
                nc.tensor.matmul(
                    ob, lhsT=wall[:, d, 8, :].bitcast(FP8), rhs=rhs,
                    start=False, stop=True
                )
                # Epilogue: out = acc - T2, fp16 (exact: integers <= 1152).
                last = d == DPC - 1
                if not (last and b == 1):
                    ofb = ofp.tile([128, 512], FP16, tag="of", name=f"of{d}{b}")
                    if b == 0:
                        nc.scalar.activation(
                            ofb, ob, Act.Identity, bias=bias, scale=1.0
                        )
                        nc.gpsimd.dma_start(od[:, b], ofb)
                    else:
                        nc.vector.tensor_scalar(
                            ofb, ob, 1.0, bias, Alu.mult, Alu.add
                        )
                        nc.sync.dma_start(od[:, b], ofb)
                else:
                    # Final block: quarters on both engines, halves on both
                    # DMA queues, to shorten the drain tail.
                    oq = od[:, 1].rearrange("c (q n) -> c q n", q=2)
                    for q in range(2):
                        ofq = ofp.tile(
                            [128, 256], FP16, tag="of", name=f"oflast{q}"
                        )
                        sl = slice(256 * q, 256 * (q + 1))
                        if q == 0:
                            nc.scalar.activation(
                                ofq, ob[:, sl], Act.Identity, bias=bias, scale=1.0
                            )
                            nc.gpsimd.dma_start(oq[:, q], ofq)
                        else:
                            nc.vector.tensor_scalar(
                                ofq, ob[:, sl], 1.0, bias, Alu.mult, Alu.add
                            )
                            nc.sync.dma_start(oq[:, q], ofq)


_NC_CACHE = None


def _get_nc():
    global _NC_CACHE
    if _NC_CACHE is None:
        nc = bacc.Bacc(
            "TRN2", target_bir_lowering=False, debug=False, num_devices=N_CORES
        )
        x_d = nc.dram_tensor(
            "x_s", [DPC, CIN, IMH, IMW], I8, kind="ExternalInput"
        ).ap()
        w_d = nc.dram_tensor(
            "w_s", [DPC, CIN, 9, COUT], I8, kind="ExternalInput"
        ).ap()
        t_d = nc.dram_tensor("t_s", [COUT, DPC], FP32, kind="ExternalInput").ap()
        o_d = nc.dram_tensor(
            "out_s", [DPC, COUT, NPIX], FP16, kind="ExternalOutput"
        ).ap()
        with tile.TileContext(nc) as tc:
            _body(nc, tc, x_d, w_d, t_d, o_d)
        nc.compile()
        _NC_CACHE = nc
    return _NC_CACHE


def _in_maps(x, w):
    # x: [D,H,W,CIN] bool -> zero-padded channel-major fp8 {0,1} image.
    xb = np.ascontiguousarray(x).view(np.uint8)  # 0/1
    xim = np.zeros((D, CIN, IMH, IMW), np.uint8)
    xim[:, :, 1 : H + 1, 1 : W + 1] = (
        np.transpose(xb, (0, 3, 1, 2)) * np.uint8(ONE_FP8)
    )
    xim = xim.view(np.int8)

    # w: [D,3,3,CIN,COUT] f32 {0,1} -> fp8 W4 = 4w-2 in [cin, tap, cout],
    # taps ordered per TAP_PERM (DoubleRow pairs adjacent).
    wb = np.ascontiguousarray(w) > 0.5
    w4 = np.where(wb, np.uint8(POS2_FP8), np.uint8(NEG2_FP8))
    perm = [3 * i + j for (i, j) in TAP_PERM]
    w4 = np.ascontiguousarray(
        np.transpose(w4.reshape(D, 9, CIN, COUT)[:, perm], (0, 2, 1, 3))
    ).view(np.int8)

    # -T2[cout] = -(2*sum(w) - K), pre-transposed to [cout, D].
    sw = wb.sum(axis=(1, 2, 3), dtype=np.int32)  # [D, COUT]
    negT = np.ascontiguousarray((9 * CIN - 2 * sw).astype(np.float32).T)

    return [
        {
            "x_s": xim[c * DPC : (c + 1) * DPC],
            "w_s": w4[c * DPC : (c + 1) * DPC],
            "t_s": negT[:, c * DPC : (c + 1) * DPC],
        }
        for c in range(N_CORES)
    ]


def kernel(x, w, _trace=False):
    nc = _get_nc()
    res = bass_utils.run_bass_kernel_spmd(
        nc, _in_maps(x, w), core_ids=list(range(N_CORES)), trace=_trace
    )
    out = np.concatenate([r["out_s"] for r in res.results], axis=0)
    # [D, COUT, NPIX] fp16 -> [D, H, W, COUT] f32 (exact: integer values)
    out = np.transpose(out, (0, 2, 1)).reshape(D, H, W, COUT).astype(np.float32)
    if _trace:
        return out, res
    return out


# revision 17
# speedup vs baseline: 1.1109x; 1.0063x over previous
# Binarized 3x3 conv (per-direction / population-parallel), Trainium2 Bass kernel.
#
# Reference math: bits {0,1} -> {-1,+1}; out = 4*xw - 2*sx - 2*sw + K.
# Identity used here:  out = conv(x, W4) - T2
#   where W4 = 4w - 2 (values +-2, exact in fp8e4), T2[cout] = sum (2w-1),
#   conv is a standard zero-padded 3x3 conv with x in {0,1}.
# Proof: sum(x*(4w-2)) - sum(2w-1) = 4xw - 2sx - (2sw - K).
# Output values are integers in [-1152, 1152] -> exact in fp16.
#
# Sharding: D=64 directions split 8 per core across 8 NeuronCores (pure
# population parallelism, no communication).
#
# All data conditioning happens on the host (not part of the HW kernel):
# x is uploaded as a zero-padded channel-major fp8 {0,1} image [34, 34],
# w as fp8 W4 = 4w-2 in [cin, tap, cout] with taps permuted so DoubleRow
# pairs are adjacent, and the -T2 bias as f32 [cout, DPC].  (Keeping x/w
# in separate SBUF tiles matters: it keeps the Tile framework's subtile
# dependency tracking precise, so the first matmuls start as soon as
# direction 0's data lands instead of waiting for every input DMA.)
#
# The device runs the conv as fp8 DoubleRow matmuls: two taps per matmul
# (2 fp8 weights per PE cell, 2x throughput), 4 pairs + 1 normal tap per
# 512-pixel block, accumulating [cout, pix] in PSUM.  The rhs pair planes
# are raw 4D access patterns over the padded image (pair stride = tap
# offset delta).  Per block: epilogue adds -T2 (ACT for block 0, DVE for
# block 1) into fp16 and DMAs out, so the block-0 epilogue hides under
# block 1's matmuls.  Scratch warmup matmuls run during the DMA fill so
# the PE clock gate (HAM, 1.2 -> 2.4 GHz after ~3.4us sustained busy) is
# warm when the real work starts.  The host transposes [cout, pix] fp16
# back to [pix, cout] f32 (exact, integer values).

import numpy as np

import concourse.bass as bass
import concourse.mybir as mybir
import concourse.tile as tile
from concourse import bacc
from concourse import bass_utils

N_CORES = 8
D, H, W, CIN, COUT = 64, 32, 32, 128, 128
DPC = D // N_CORES  # directions per core
NPIX = H * W  # 1024
IMH, IMW = 34, 34  # padded image
IMSZ = IMH * IMW  # 1156
WSZ = 9 * COUT  # 1152

FP32 = mybir.dt.float32
FP16 = mybir.dt.float16
BF16 = mybir.dt.bfloat16
FP8 = mybir.dt.float8e4
I8 = mybir.dt.int8

ONE_FP8 = 0x38  # 1.0 in e4m3
POS2_FP8 = 0x40  # 2.0
NEG2_FP8 = 0xC0  # -2.0

# Tap order in the uploaded weight buffer: DoubleRow pairs adjacent.
# (i, j) = (filter row, filter col); window offset in image = i*34 + j.
TAP_PERM = [(0, 0), (0, 1), (1, 0), (1, 1), (2, 0), (2, 1), (0, 2), (1, 2), (2, 2)]
N_WARMUP = 4


def _body(nc, tc, x_d, w_d, t_d, o_d):
    Act = mybir.ActivationFunctionType
    Alu = mybir.AluOpType
    DR = mybir.MatmulPerfMode.DoubleRow
    with (
        tc.tile_pool(name="const", bufs=1) as constp,
        tc.tile_pool(name="of", bufs=2 * DPC, space="SBUF") as ofp,
        tc.tile_pool(name="psA", bufs=4, space="PSUM") as psA,
        tc.tile_pool(name="psW", bufs=1, space="PSUM") as psW,
    ):
        # PE warmup: HAM un-throttles (1.2 -> 2.4 GHz) only after ~3.4us of
        # sustained matmul activity; burn the DMA-fill window on scratch
        # matmuls so the real ones run warm.
        scratch = constp.tile([128, 512], BF16)
        nc.vector.memset(scratch, 0.0)
        wacc = psW.tile([128, 512], FP32)
        for _ in range(N_WARMUP):
            nc.tensor.matmul(
                wacc, lhsT=scratch[:, 0:128], rhs=scratch, start=True, stop=True
            )

        # All input DMAs issued upfront (SBUF easily fits every direction).
        # Direction 0 is split into halves across the two HWDGE queues
        # (sync/scalar) so its descgen + transfer lands soonest; gpsimd's
        # queue starts later, so it only carries the tiny bias load and
        # the block-0 output DMAs.
        xall = constp.tile([128, DPC, IMSZ], I8)
        wall = constp.tile([128, DPC, 9, COUT], I8)
        negT = constp.tile([128, DPC], FP32)
        x0 = x_d[0].rearrange("c h w -> c (h w)")
        w0 = w_d[0].rearrange("c t o -> c (t o)")
        x0sb = xall[:, 0]
        w0sb = wall[:, 0].rearrange("p t o -> p (t o)")
        nc.sync.dma_start(x0sb[:, 0:578], x0[:, 0:578])
        nc.scalar.dma_start(w0sb[:, 0:576], w0[:, 0:576])
        nc.sync.dma_start(w0sb[:, 576:1152], w0[:, 576:1152])
        nc.scalar.dma_start(x0sb[:, 578:1156], x0[:, 578:1156])
        nc.gpsimd.dma_start(negT, t_d)
        for d in range(1, DPC):
            nc.sync.dma_start(xall[:, d], x_d[d].rearrange("c h w -> c (h w)"))
            nc.scalar.dma_start(wall[:, d], w_d[d])

        for d in range(DPC):
            xim = xall[:, d].bitcast(FP8)
            pstride = xim.ap[0]
            bias = negT[:, d : d + 1]
            od = o_d[d].rearrange("c (b n) -> c b n", b=2)
            # 9-tap conv: out[cout, pix] += W4[tap].T @ xim[window], as 4
            # DoubleRow pair-matmuls + 1 normal per 512-pixel block.  The
            # rhs pair AP reads both taps' windows (2nd plane at +delta).
            for b in range(2):
                ob = psA.tile([128, 512], FP32, tag="acc", name=f"acc{d}{b}")
                for k in range(4):
                    (i0, j0), (i1, j1) = TAP_PERM[2 * k], TAP_PERM[2 * k + 1]
                    off = (16 * b + i0) * IMW + j0
                    delta = (i1 - i0) * IMW + (j1 - j0)
                    rhs = bass.AP(
                        xim.tensor,
                        xim.offset + off,
                        [pstride, [delta, 2], [IMW, 16], [1, 32]],
                    )
                    nc.tensor.matmul(
                        ob,
                        lhsT=wall[:, d, 2 * k : 2 * k + 2, :].bitcast(FP8),
                        rhs=rhs,
                        start=(k == 0), stop=False, perf_mode=DR,
                    )
                i8, j8 = TAP_PERM[8]
                off = (16 * b + i8) * IMW + j8
                rhs = bass.AP(
                    xim.tensor, xim.offset + off, [pstride, [IMW, 16], [1, 32]]
                )
                nc.tensor.matmul(
                    ob, lhsT=wall[:, d, 8, :].bitcast(FP8), rhs=rhs,
                    start=False, stop=True
                )
                # Epilogue: out = acc - T2, fp16 (exact: integers <= 1152).
                last = d == DPC - 1
                if not (last and b == 1):
                    ofb = ofp.tile([128, 512], FP16, tag="of", name=f"of{d}{b}")
                    if b == 0:
                        nc.scalar.activation(
                            ofb, ob, Act.Identity, bias=bias, scale=1.0
                        )
                        nc.gpsimd.dma_start(od[:, b], ofb)
                    else:
                        nc.vector.tensor_scalar(
                            ofb, ob, 1.0, bias, Alu.mult, Alu.add
                        )
                        nc.sync.dma_start(od[:, b], ofb)
                else:
                    # Final block: two sequential DVE chunks, each chased
                    # by its half-DMA on its own queue, so the first
                    # half's descgen+transfer hides under the second
                    # chunk (readers of one PSUM tile serialize anyway).
                    oq = od[:, 1].rearrange("c (q n) -> c q n", q=2)
                    for q in range(2):
                        ofq = ofp.tile(
                            [128, 256], FP16, tag="of", name=f"oflast{q}"
                        )
                        sl = slice(256 * q, 256 * (q + 1))
                        nc.vector.tensor_scalar(
                            ofq, ob[:, sl], 1.0, bias, Alu.mult, Alu.add
                        )
                        dq = nc.sync if q == 0 else nc.gpsimd
                        dq.dma_start(oq[:, q], ofq)


_NC_CACHE = None


def _get_nc():
    global _NC_CACHE
    if _NC_CACHE is None:
        nc = bacc.Bacc(
            "TRN2", target_bir_lowering=False, debug=False, num_devices=N_CORES
        )
        x_d = nc.dram_tensor(
            "x_s", [DPC, CIN, IMH, IMW], I8, kind="ExternalInput"
        ).ap()
        w_d = nc.dram_tensor(
            "w_s", [DPC, CIN, 9, COUT], I8, kind="ExternalInput"
        ).ap()
        t_d = nc.dram_tensor("t_s", [COUT, DPC], FP32, kind="ExternalInput").ap()
        o_d = nc.dram_tensor(
            "out_s", [DPC, COUT, NPIX], FP16, kind="ExternalOutput"
        ).ap()
        with tile.TileContext(nc) as tc:
            _body(nc, tc, x_d, w_d, t_d, o_d)
        nc.compile()
        _NC_CACHE = nc
    return _NC_CACHE


def _in_maps(x, w):
    # x: [D,H,W,CIN] bool -> zero-padded channel-major fp8 {0,1} image.
    xb = np.ascontiguousarray(x).view(np.uint8)  # 0/1
    xim = np.zeros((D, CIN, IMH, IMW), np.uint8)
    xim[:, :, 1 : H + 1, 1 : W + 1] = (
        np.transpose(xb, (0, 3, 1, 2)) * np.uint8(ONE_FP8)
    )
    xim = xim.view(np.int8)

    # w: [D,3,3,CIN,COUT] f32 {0,1} -> fp8 W4 = 4w-2 in [cin, tap, cout],
    # taps ordered per TAP_PERM (DoubleRow pairs adjacent).
    wb = np.ascontiguousarray(w) > 0.5
    w4 = np.where(wb, np.uint8(POS2_FP8), np.uint8(NEG2_FP8))
    perm = [3 * i + j for (i, j) in TAP_PERM]
    w4 = np.ascontiguousarray(
        np.transpose(w4.reshape(D, 9, CIN, COUT)[:, perm], (0, 2, 1, 3))
    ).view(np.int8)

    # -T2[cout] = -(2*sum(w) - K), pre-transposed to [cout, D].
    sw = wb.sum(axis=(1, 2, 3), dtype=np.int32)  # [D, COUT]
    negT = np.ascontiguousarray((9 * CIN - 2 * sw).astype(np.float32).T)

    return [
        {
            "x_s": xim[c * DPC : (c + 1) * DPC],
            "w_s": w4[c * DPC : (c + 1) * DPC],
            "t_s": negT[:, c * DPC : (c + 1) * DPC],
        }
        for c in range(N_CORES)
    ]


def kernel(x, w, _trace=False):
    nc = _get_nc()
    res = bass_utils.run_bass_kernel_spmd(
        nc, _in_maps(x, w), core_ids=list(range(N_CORES)), trace=_trace
    )
    out = np.concatenate([r["out_s"] for r in res.results], axis=0)
    # [D, COUT, NPIX] fp16 -> [D, H, W, COUT] f32 (exact: integer values)
    out = np.transpose(out, (0, 2, 1)).reshape(D, H, W, COUT).astype(np.float32)
    if _trace:
        return out, res
    return out


# revision 18
# speedup vs baseline: 1.1156x; 1.0042x over previous
# Binarized 3x3 conv (per-direction / population-parallel), Trainium2 Bass kernel.
#
# Reference math: bits {0,1} -> {-1,+1}; out = 4*xw - 2*sx - 2*sw + K.
# Identity used here:  out = conv(x, W4) - T2
#   where W4 = 4w - 2 (values +-2, exact in fp8e4), T2[cout] = sum (2w-1),
#   conv is a standard zero-padded 3x3 conv with x in {0,1}.
# Proof: sum(x*(4w-2)) - sum(2w-1) = 4xw - 2sx - (2sw - K).
# Output values are integers in [-1152, 1152] -> exact in fp16.
#
# Sharding: D=64 directions split 8 per core across 8 NeuronCores (pure
# population parallelism, no communication).
#
# All data conditioning happens on the host (not part of the HW kernel):
# x is uploaded as a zero-padded channel-major fp8 {0,1} image [34, 34],
# w as fp8 W4 = 4w-2 in [cin, tap, cout] with taps permuted so DoubleRow
# pairs are adjacent, and the -T2 bias as f32 [cout, DPC].  (Keeping x/w
# in separate SBUF tiles matters: it keeps the Tile framework's subtile
# dependency tracking precise, so the first matmuls start as soon as
# direction 0's data lands instead of waiting for every input DMA.)
#
# The device runs the conv as fp8 DoubleRow matmuls: two taps per matmul
# (2 fp8 weights per PE cell, 2x throughput), 4 pairs + 1 normal tap per
# 512-pixel block, accumulating [cout, pix] in PSUM.  The rhs pair planes
# are raw 4D access patterns over the padded image (pair stride = tap
# offset delta).  Per block: epilogue adds -T2 (ACT for block 0, DVE for
# block 1) into fp16 and DMAs out, so the block-0 epilogue hides under
# block 1's matmuls.  Scratch warmup matmuls run during the DMA fill so
# the PE clock gate (HAM, 1.2 -> 2.4 GHz after ~3.4us sustained busy) is
# warm when the real work starts.  The host transposes [cout, pix] fp16
# back to [pix, cout] f32 (exact, integer values).

import numpy as np

import concourse.bass as bass
import concourse.mybir as mybir
import concourse.tile as tile
from concourse import bacc
from concourse import bass_utils

N_CORES = 8
D, H, W, CIN, COUT = 64, 32, 32, 128, 128
DPC = D // N_CORES  # directions per core
NPIX = H * W  # 1024
IMH, IMW = 34, 34  # padded image
IMSZ = IMH * IMW  # 1156
WSZ = 9 * COUT  # 1152

FP32 = mybir.dt.float32
FP16 = mybir.dt.float16
BF16 = mybir.dt.bfloat16
FP8 = mybir.dt.float8e4
I8 = mybir.dt.int8

ONE_FP8 = 0x38  # 1.0 in e4m3
POS2_FP8 = 0x40  # 2.0
NEG2_FP8 = 0xC0  # -2.0

# Tap order in the uploaded weight buffer: DoubleRow pairs adjacent.
# (i, j) = (filter row, filter col); window offset in image = i*34 + j.
TAP_PERM = [(0, 0), (0, 1), (1, 0), (1, 1), (2, 0), (2, 1), (0, 2), (1, 2), (2, 2)]
N_WARMUP = 4


def _body(nc, tc, x_d, w_d, t_d, o_d):
    Act = mybir.ActivationFunctionType
    Alu = mybir.AluOpType
    DR = mybir.MatmulPerfMode.DoubleRow
    with (
        tc.tile_pool(name="const", bufs=1) as constp,
        tc.tile_pool(name="of", bufs=2 * DPC, space="SBUF") as ofp,
        tc.tile_pool(name="psA", bufs=4, space="PSUM") as psA,
        tc.tile_pool(name="psW", bufs=1, space="PSUM") as psW,
    ):
        # PE warmup: HAM un-throttles (1.2 -> 2.4 GHz) only after ~3.4us of
        # sustained matmul activity; burn the DMA-fill window on scratch
        # matmuls so the real ones run warm.
        scratch = constp.tile([128, 512], BF16)
        nc.vector.memset(scratch, 0.0)
        wacc = psW.tile([128, 512], FP32)
        for _ in range(N_WARMUP):
            nc.tensor.matmul(
                wacc, lhsT=scratch[:, 0:128], rhs=scratch, start=True, stop=True
            )

        # All input DMAs issued upfront (SBUF easily fits every direction).
        # Direction 0 is split into halves across the two HWDGE queues
        # (sync/scalar) so its descgen + transfer lands soonest; gpsimd's
        # queue starts later, so it only carries the tiny bias load and
        # the block-0 output DMAs.
        xall = constp.tile([128, DPC, IMSZ], I8)
        wall = constp.tile([128, DPC, 9, COUT], I8)
        negT = constp.tile([128, DPC], FP32)
        x0 = x_d[0].rearrange("c h w -> c (h w)")
        w0 = w_d[0].rearrange("c t o -> c (t o)")
        x0sb = xall[:, 0]
        w0sb = wall[:, 0].rearrange("p t o -> p (t o)")
        nc.sync.dma_start(x0sb[:, 0:578], x0[:, 0:578])
        nc.scalar.dma_start(w0sb[:, 0:576], w0[:, 0:576])
        nc.sync.dma_start(w0sb[:, 576:1152], w0[:, 576:1152])
        nc.scalar.dma_start(x0sb[:, 578:1156], x0[:, 578:1156])
        nc.gpsimd.dma_start(negT, t_d)
        for d in range(1, DPC):
            nc.sync.dma_start(xall[:, d], x_d[d].rearrange("c h w -> c (h w)"))
            nc.scalar.dma_start(wall[:, d], w_d[d])

        for d in range(DPC):
            xim = xall[:, d].bitcast(FP8)
            pstride = xim.ap[0]
            bias = negT[:, d : d + 1]
            od = o_d[d].rearrange("c (b n) -> c b n", b=2)
            # 9-tap conv: out[cout, pix] += W4[tap].T @ xim[window], as 4
            # DoubleRow pair-matmuls + 1 normal per 512-pixel block.  The
            # rhs pair AP reads both taps' windows (2nd plane at +delta).
            for b in range(2):
                ob = psA.tile([128, 512], FP32, tag="acc", name=f"acc{d}{b}")
                for k in range(4):
                    (i0, j0), (i1, j1) = TAP_PERM[2 * k], TAP_PERM[2 * k + 1]
                    off = (16 * b + i0) * IMW + j0
                    delta = (i1 - i0) * IMW + (j1 - j0)
                    rhs = bass.AP(
                        xim.tensor,
                        xim.offset + off,
                        [pstride, [delta, 2], [IMW, 16], [1, 32]],
                    )
                    nc.tensor.matmul(
                        ob,
                        lhsT=wall[:, d, 2 * k : 2 * k + 2, :].bitcast(FP8),
                        rhs=rhs,
                        start=(k == 0), stop=False, perf_mode=DR,
                    )
                i8, j8 = TAP_PERM[8]
                off = (16 * b + i8) * IMW + j8
                rhs = bass.AP(
                    xim.tensor, xim.offset + off, [pstride, [IMW, 16], [1, 32]]
                )
                nc.tensor.matmul(
                    ob, lhsT=wall[:, d, 8, :].bitcast(FP8), rhs=rhs,
                    start=False, stop=True
                )
                # Epilogue: out = acc - T2, fp16 (exact: integers <= 1152).
                last = d == DPC - 1
                if not (last and b == 1):
                    ofb = ofp.tile([128, 512], FP16, tag="of", name=f"of{d}{b}")
                    if b == 0:
                        nc.scalar.activation(
                            ofb, ob, Act.Identity, bias=bias, scale=1.0
                        )
                        nc.gpsimd.dma_start(od[:, b], ofb)
                    else:
                        nc.vector.tensor_scalar(
                            ofb, ob, 1.0, bias, Alu.mult, Alu.add
                        )
                        nc.sync.dma_start(od[:, b], ofb)
                else:
                    # Final block: one ACT epilogue (fastest single op on
                    # a PSUM read; two ops on one PSUM tile would chain),
                    # then two half-DMAs descgen in parallel on both
                    # queues to shorten the drain tail.
                    ofb = ofp.tile([128, 512], FP16, tag="of", name="oflast")
                    nc.scalar.activation(
                        ofb, ob, Act.Identity, bias=bias, scale=1.0
                    )
                    oq = od[:, 1].rearrange("c (q n) -> c q n", q=2)
                    of2 = ofb.rearrange("c (q n) -> c q n", q=2)
                    nc.sync.dma_start(oq[:, 0], of2[:, 0])
                    nc.gpsimd.dma_start(oq[:, 1], of2[:, 1])


_NC_CACHE = None


def _get_nc():
    global _NC_CACHE
    if _NC_CACHE is None:
        nc = bacc.Bacc(
            "TRN2", target_bir_lowering=False, debug=False, num_devices=N_CORES
        )
        x_d = nc.dram_tensor(
            "x_s", [DPC, CIN, IMH, IMW], I8, kind="ExternalInput"
        ).ap()
        w_d = nc.dram_tensor(
            "w_s", [DPC, CIN, 9, COUT], I8, kind="ExternalInput"
        ).ap()
        t_d = nc.dram_tensor("t_s", [COUT, DPC], FP32, kind="ExternalInput").ap()
        o_d = nc.dram_tensor(
            "out_s", [DPC, COUT, NPIX], FP16, kind="ExternalOutput"
        ).ap()
        with tile.TileContext(nc) as tc:
            _body(nc, tc, x_d, w_d, t_d, o_d)
        nc.compile()
        _NC_CACHE = nc
    return _NC_CACHE


def _in_maps(x, w):
    # x: [D,H,W,CIN] bool -> zero-padded channel-major fp8 {0,1} image.
    xb = np.ascontiguousarray(x).view(np.uint8)  # 0/1
    xim = np.zeros((D, CIN, IMH, IMW), np.uint8)
    xim[:, :, 1 : H + 1, 1 : W + 1] = (
        np.transpose(xb, (0, 3, 1, 2)) * np.uint8(ONE_FP8)
    )
    xim = xim.view(np.int8)

    # w: [D,3,3,CIN,COUT] f32 {0,1} -> fp8 W4 = 4w-2 in [cin, tap, cout],
    # taps ordered per TAP_PERM (DoubleRow pairs adjacent).
    wb = np.ascontiguousarray(w) > 0.5
    w4 = np.where(wb, np.uint8(POS2_FP8), np.uint8(NEG2_FP8))
    perm = [3 * i + j for (i, j) in TAP_PERM]
    w4 = np.ascontiguousarray(
        np.transpose(w4.reshape(D, 9, CIN, COUT)[:, perm], (0, 2, 1, 3))
    ).view(np.int8)

    # -T2[cout] = -(2*sum(w) - K), pre-transposed to [cout, D].
    sw = wb.sum(axis=(1, 2, 3), dtype=np.int32)  # [D, COUT]
    negT = np.ascontiguousarray((9 * CIN - 2 * sw).astype(np.float32).T)

    return [
        {
            "x_s": xim[c * DPC : (c + 1) * DPC],
            "w_s": w4[c * DPC : (c + 1) * DPC],
            "t_s": negT[:, c * DPC : (c + 1) * DPC],
        }
        for c in range(N_CORES)
    ]


def kernel(x, w, _trace=False):
    nc = _get_nc()
    res = bass_utils.run_bass_kernel_spmd(
        nc, _in_maps(x, w), core_ids=list(range(N_CORES)), trace=_trace
    )
    out = np.concatenate([r["out_s"] for r in res.results], axis=0)
    # [D, COUT, NPIX] fp16 -> [D, H, W, COUT] f32 (exact: integer values)
    out = np.transpose(out, (0, 2, 1)).reshape(D, H, W, COUT).astype(np.float32)
    if _trace:
        return out, res
    return out
